# revision 1
# baseline (speedup 1.0000x reference)
"""DeepCrossing (embedding bag lookup + residual MLP) Trainium2 kernel.

Strategy (8 NeuronCores, data-parallel over batch):
  - Batch 4096 split 512 samples/core; the 1M x 64 embedding table is
    replicated per core as bf16 padded to 256B rows ([1M, 128] bf16).
  - Gather is 3-hop to dodge two hardware limits (indirect DMA = 1 offset
    per partition at ~1us/op; dma_gather is fast per-index but int16):
      hop1: per vocab bank of 32768 rows, one dma_gather (bank-local int16
            idx, slot-sorted, fixed 1024-slot capacity per (quarter, bank))
      hop2: HWDGE strided write to DRAM scratch [31744, 128] bf16 per
            quarter of 128 samples (window < 32768 so hop3 idx fit int16)
      hop3: per (quarter, feature-pair) dma_gather from scratch in SLOT
            order -> [128 slots, 64] bf16 tiles
    Gathers use a raw InstDMAGatherAnt constructor: 128B payload with 256B
    row stride (elem_size=64 bf16, elem_step=128) — HW-validated.
  - Pooling: PE matmuls out[64,16] = G.T @ S (S = 0/1 bag matrix) write the
    TRANSPOSED feature matrix featT [1664, 512] directly.
  - 3 residual blocks + final linear as bf16 matmuls (f32 PSUM); bias+ReLU
    on scalar engine, residual adds on vector engine; sigmoid -> [512]/core.

Self-contained: hardcodes problem shapes from the task spec.
"""

import numpy as np
import ml_dtypes

# ---- problem constants ----
V = 1_000_000
D = 64
F = 26
L = 8
B = 4096
NCORES = 8
IN_DIM = F * D  # 1664
HIDDENS = (1024, 1024, 512)
P = 128
QS = 128                       # samples per quarter
BF16 = ml_dtypes.bfloat16


def _ceil(a, b):
    return -(-a // b)


# --------------------------------------------------------------------
# Host-side gather planning (per core); fixed capacities so one bass
# program serves every core.
# --------------------------------------------------------------------
def plan_gather(x_core, V_=V, F_=F, L_=L, SPC=None, bank=32768, qcap=1024):
    SPC = SPC or x_core.shape[0]
    nbank = _ceil(V_, bank)
    nq = SPC // QS
    spq = F_ * QS * L_                        # slots per quarter
    xq = x_core.reshape(nq, QS, F_, L_)       # [q, s, f, l]
    ids = np.ascontiguousarray(xq.transpose(0, 2, 1, 3)).reshape(nq, spq)
    bk = ids // bank
    local = (ids % bank).astype(np.int16)

    flat1 = np.zeros(nbank * nq * qcap, dtype=np.int16)
    idx3 = np.zeros((nq, spq), dtype=np.int16)
    for b in range(nbank):
        for q in range(nq):
            sel = np.where(bk[q] == b)[0]
            n = len(sel)
            assert n <= qcap, f"bank overflow: {n} > {qcap}"
            o = (b * nq + q) * qcap
            flat1[o:o + n] = local[q, sel]
            if n:
                flat1[o + n:o + qcap] = local[q, sel[-1]]
            idx3[q, sel] = (b * qcap + np.arange(n)).astype(np.int16)

    def wrap(flat):
        n = len(flat)
        w = np.zeros((16, n // 16), dtype=np.int16)
        w[np.arange(n) % 16, np.arange(n) // 16] = flat
        return np.ascontiguousarray(np.tile(w, (8, 1)))

    return {"idx1": wrap(flat1), "idx3": wrap(idx3.reshape(-1)),
            "nq": nq, "spq": spq, "nbank": nbank, "bank": bank, "qcap": qcap}


# --------------------------------------------------------------------
# Raw dma_gather: 128B payload + 256B stride (skips the %256 payload
# assert; stride encoding still requires multiples of 256B).
# --------------------------------------------------------------------
def _dma_gather_raw(gp, mybir, out_ap, in_ap, idxs_ap, num_idxs,
                    elem_size, elem_step):
    assert idxs_ap.dtype == mybir.dt.int16
    assert in_ap.dtype == out_ap.dtype
    stride_bytes = elem_step * mybir.dt.size(in_ap.dtype)
    stride_bytes_256 = stride_bytes // 256
    assert stride_bytes_256 * 256 == stride_bytes and stride_bytes_256 < 256
    assert in_ap.ap[-1][1] == out_ap.ap[-1][1] == elem_size
    assert in_ap.ap[0][0] == elem_step
    _in_ap = gp.lower_ap_dma(in_ap, for_custom_bir_dma=True)
    _idxs_ap = gp.lower_ap(idxs_ap)
    _out_ap = gp.lower_ap(out_ap)
    return gp.add_instruction(
        mybir.InstDMAGatherAnt(
            name=gp.bass.get_next_instruction_name(),
            ins=[*_in_ap, _idxs_ap, gp.lower_val_access(gp.to_reg(num_idxs))],
            outs=[_out_ap],
            transpose=False,
            num_idxs=num_idxs,
            elem_size=elem_size,
            stride_bytes_256=stride_bytes_256,
            gen_mode=0,
            single_packet=False,
            queue_num=0,
            sbuf_tokens_per_rank=0,
            sbuf_free_dim_per_rank=0,
            sbuf_free_dim_pad_per_rank=0,
            sbuf_byte_offset=0,
        ))


# --------------------------------------------------------------------
# Bass kernel builder
# --------------------------------------------------------------------
def build_nc(plan, V_=V, F_=F, SPC=B // NCORES, hiddens=HIDDENS, stage=9):
    from contextlib import ExitStack

    from concourse import bacc, mybir, tile

    IN = F_ * D
    KT = IN // P
    nq, nbank = plan["nq"], plan["nbank"]
    bank, qcap = plan["bank"], plan["qcap"]
    spq = plan["spq"]
    n1cols = plan["idx1"].shape[1]
    n3cols = plan["idx3"].shape[1]
    opn1 = nq * qcap                      # idxs per hop1 op
    scrr = nbank * qcap                   # scratch rows per quarter
    MTs = [h // P for h in hiddens]
    bf16 = mybir.dt.bfloat16
    f32 = mybir.dt.float32
    i16 = mybir.dt.int16
    AF = mybir.ActivationFunctionType
    ALU = mybir.AluOpType

    nc = bacc.Bacc("TRN2", target_bir_lowering=False, debug=False)

    emb = nc.dram_tensor("emb", [V_, 2 * D], bf16, kind="ExternalInput")
    idx1 = nc.dram_tensor("idx1", [P, n1cols], i16, kind="ExternalInput")
    idx3 = nc.dram_tensor("idx3", [P, n3cols], i16, kind="ExternalInput")
    S_in = nc.dram_tensor("S", [P, 16], bf16, kind="ExternalInput")
    w1d = [nc.dram_tensor(f"w1_{i}", [P, KT * h], bf16, kind="ExternalInput")
           for i, h in enumerate(hiddens)]
    w2d = [nc.dram_tensor(f"w2_{i}", [P, (h // P) * IN], bf16, kind="ExternalInput")
           for i, h in enumerate(hiddens)]
    b1d = nc.dram_tensor("b1", [P, sum(MTs)], f32, kind="ExternalInput")
    b2d = nc.dram_tensor("b2", [P, len(hiddens) * KT], f32, kind="ExternalInput")
    lwd = nc.dram_tensor("lin_w", [P, KT], bf16, kind="ExternalInput")
    lbd = nc.dram_tensor("lin_b", [1, 1], f32, kind="ExternalInput")
    out_d = nc.dram_tensor("out", [1, SPC], f32, kind="ExternalOutput")

    with tile.TileContext(nc) as tc, ExitStack() as ctx:
        const = ctx.enter_context(tc.tile_pool(name="const", bufs=1))
        g1pool = ctx.enter_context(tc.tile_pool(name="g1", bufs=4))
        g3pool = ctx.enter_context(tc.tile_pool(name="g3", bufs=4))
        wpool = ctx.enter_context(tc.tile_pool(name="w", bufs=2))
        apool = ctx.enter_context(tc.tile_pool(name="acts", bufs=1))
        tpool = ctx.enter_context(tc.tile_pool(name="tmp", bufs=2))
        scrp = ctx.enter_context(tc.tile_pool(name="scr", bufs=1, space="DRAM"))
        fps_p = ctx.enter_context(tc.tile_pool(name="fps", bufs=4, space="PSUM"))
        mm_p = ctx.enter_context(tc.tile_pool(name="mm", bufs=2, space="PSUM"))
        op_p = ctx.enter_context(tc.tile_pool(name="op", bufs=1, space="PSUM"))

        idx1_sb = const.tile([P, n1cols], i16)
        idx3_sb = const.tile([P, n3cols], i16)
        S_sb = const.tile([P, 16], bf16)
        b1_sb = const.tile([P, sum(MTs)], f32)
        b2_sb = const.tile([P, len(hiddens) * KT], f32)
        lw_sb = const.tile([P, KT], bf16)
        lb_sb = const.tile([1, 1], f32)
        nc.sync.dma_start(out=idx1_sb[:], in_=idx1[:])
        nc.sync.dma_start(out=idx3_sb[:], in_=idx3[:])
        nc.sync.dma_start(out=S_sb[:], in_=S_in[:])
        nc.sync.dma_start(out=b1_sb[:], in_=b1d[:])
        nc.sync.dma_start(out=b2_sb[:], in_=b2d[:])
        nc.sync.dma_start(out=lw_sb[:], in_=lwd[:])
        nc.sync.dma_start(out=lb_sb[:], in_=lbd[:])

        featT = apool.tile([P, KT * SPC], bf16)
        hT = apool.tile([P, max(MTs) * SPC], bf16)
        out_sb = apool.tile([1, SPC], f32)

        w1_sb = []
        w2_sb = []
        for i, h in enumerate(hiddens):
            t1 = wpool.tile([P, KT * h], bf16, tag="w1")
            nc.sync.dma_start(out=t1[:], in_=w1d[i][:])
            w1_sb.append(t1)
            t2 = wpool.tile([P, MTs[i] * IN], bf16, tag="w2")
            nc.sync.dma_start(out=t2[:], in_=w2d[i][:])
            w2_sb.append(t2)

        scr = [scrp.tile([scrr, 2 * D], bf16, tag=f"scr{q}", name=f"scr{q}")
               for q in range(nq)]

        # ---- hop1 + hop2, per bank ----
        for b in range(nbank):
            bank_rows = min(bank, V_ - b * bank)
            g1 = g1pool.tile([P, opn1 // P, D], bf16, tag="g1")
            _dma_gather_raw(
                nc.gpsimd, mybir, g1[:, :, :],
                emb[b * bank: b * bank + bank_rows, 0:D],
                idx1_sb[:, b * (opn1 // 16):(b + 1) * (opn1 // 16)],
                num_idxs=opn1, elem_size=D, elem_step=2 * D)
            if stage < 2:
                if b == 0:
                    nc.vector.tensor_copy(out=featT[:, 0:D], in_=g1[:, 0, :])
                continue
            for q in range(nq):
                dst = scr[q][b * qcap:(b + 1) * qcap, 0:D].rearrange(
                    "(blk p) d -> p blk d", p=P)
                eng = nc.sync if (b % 2 == 0) else nc.scalar
                eng.dma_start(
                    out=dst, in_=g1[:, q * (qcap // P):(q + 1) * (qcap // P), :])

        # ---- hop3 + pooling ----
        CHUNK = 2 * QS * L                  # 2048 slots = 2 features
        if stage >= 3:
            for q in range(nq):
                for kt in range(KT):
                    g3 = g3pool.tile([P, CHUNK // P, D], bf16, tag="g3")
                    c0 = (q * spq + kt * CHUNK) // 16
                    _dma_gather_raw(
                        nc.gpsimd, mybir, g3[:, :, :], scr[q][:, 0:D],
                        idx3_sb[:, c0:c0 + CHUNK // 16],
                        num_idxs=CHUNK, elem_size=D, elem_step=2 * D)
                    fps = fps_p.tile([P, QS], f32, tag="fps")
                    for blk in range(CHUNK // P):
                        f_loc = blk // 8
                        j = blk % 8
                        nc.tensor.matmul(
                            out=fps[f_loc * 64:(f_loc + 1) * 64,
                                    j * 16:(j + 1) * 16],
                            lhsT=g3[:, blk, :],
                            rhs=S_sb[:],
                            start=True, stop=True)
                    nc.vector.tensor_copy(
                        out=featT[:, kt * SPC + q * QS: kt * SPC + (q + 1) * QS],
                        in_=fps[:])

        # ---- residual MLP ----
        b1_off = 0
        for i, h in enumerate(hiddens):
            MT = MTs[i]
            for m in range(MT):
                ps = mm_p.tile([P, SPC], f32, tag="mm")
                for k in range(KT):
                    nc.tensor.matmul(
                        out=ps[:],
                        lhsT=w1_sb[i][:, k * h + m * P: k * h + (m + 1) * P],
                        rhs=featT[:, k * SPC:(k + 1) * SPC],
                        start=(k == 0), stop=(k == KT - 1))
                nc.scalar.activation(
                    out=hT[:, m * SPC:(m + 1) * SPC], in_=ps[:], func=AF.Relu,
                    bias=b1_sb[:, b1_off + m: b1_off + m + 1])
            b1_off += MT
            for k in range(KT):
                ps = mm_p.tile([P, SPC], f32, tag="mm")
                for m in range(MT):
                    nc.tensor.matmul(
                        out=ps[:],
                        lhsT=w2_sb[i][:, m * IN + k * P: m * IN + (k + 1) * P],
                        rhs=hT[:, m * SPC:(m + 1) * SPC],
                        start=(m == 0), stop=(m == MT - 1))
                tmp = tpool.tile([P, SPC], f32, tag="tmp")
                nc.vector.scalar_tensor_tensor(
                    out=tmp[:], in0=ps[:],
                    scalar=b2_sb[:, i * KT + k: i * KT + k + 1],
                    in1=featT[:, k * SPC:(k + 1) * SPC],
                    op0=ALU.add, op1=ALU.add)
                nc.scalar.activation(
                    out=featT[:, k * SPC:(k + 1) * SPC], in_=tmp[:], func=AF.Relu)

        ps = op_p.tile([1, SPC], f32, tag="op")
        for k in range(KT):
            nc.tensor.matmul(
                out=ps[:], lhsT=lw_sb[:, k:k + 1],
                rhs=featT[:, k * SPC:(k + 1) * SPC],
                start=(k == 0), stop=(k == KT - 1))
        nc.scalar.activation(out=out_sb[:], in_=ps[:], func=AF.Sigmoid,
                             bias=lb_sb[0:1, 0:1])
        nc.sync.dma_start(out=out_d[:], in_=out_sb[:])

    nc.compile()
    return nc


# --------------------------------------------------------------------
# Host-side input prep
# --------------------------------------------------------------------
def _prep_weights(inputs, hiddens=HIDDENS, F_=F):
    IN = F_ * D
    KT = IN // P
    MTs = [h // P for h in hiddens]
    embf = np.asarray(inputs["emb_table"], dtype=np.float32)
    embp = np.zeros((embf.shape[0], 2 * D), dtype=BF16)
    embp[:, :D] = embf.astype(BF16)
    shared = {
        "emb": embp,
        "S": (np.arange(P)[:, None] // L == np.arange(16)[None, :]).astype(BF16),
    }
    for i, h in enumerate(hiddens):
        w1 = np.asarray(inputs[f"w1_{i}"], dtype=np.float32)
        w2 = np.asarray(inputs[f"w2_{i}"], dtype=np.float32)
        shared[f"w1_{i}"] = np.ascontiguousarray(
            w1.reshape(KT, P, h).transpose(1, 0, 2).reshape(P, KT * h).astype(BF16))
        shared[f"w2_{i}"] = np.ascontiguousarray(
            w2.reshape(h // P, P, IN).transpose(1, 0, 2)
            .reshape(P, (h // P) * IN).astype(BF16))
    b1 = np.concatenate([np.asarray(inputs[f"b1_{i}"], dtype=np.float32)
                         .reshape(MTs[i], P).T for i in range(len(hiddens))],
                        axis=1)
    b2 = np.concatenate([np.asarray(inputs[f"b2_{i}"], dtype=np.float32)
                         .reshape(KT, P).T for i in range(len(hiddens))], axis=1)
    shared["b1"] = np.ascontiguousarray(b1)
    shared["b2"] = np.ascontiguousarray(b2)
    shared["lin_w"] = np.ascontiguousarray(
        np.asarray(inputs["lin_w"], dtype=np.float32).reshape(KT, P).T.astype(BF16))
    shared["lin_b"] = np.asarray(inputs["lin_b"], dtype=np.float32).reshape(1, 1)
    return shared


# --------------------------------------------------------------------
# Public entry point
# --------------------------------------------------------------------
_NC_CACHE = {}


def kernel(**inputs):
    from concourse.bass_utils import run_bass_kernel_spmd

    SPC = B // NCORES
    x = np.asarray(inputs["x"]).astype(np.int64)
    shared = _prep_weights(inputs)
    plans = [plan_gather(x[c * SPC:(c + 1) * SPC]) for c in range(NCORES)]
    if "nc" not in _NC_CACHE:
        _NC_CACHE["nc"] = build_nc(plans[0])
    nc = _NC_CACHE["nc"]
    in_maps = []
    for c in range(NCORES):
        m = dict(shared)
        m["idx1"] = plans[c]["idx1"]
        m["idx3"] = plans[c]["idx3"]
        in_maps.append(m)
    res = run_bass_kernel_spmd(nc, in_maps, core_ids=list(range(NCORES)))
    outs = [np.asarray(r["out"], dtype=np.float32).reshape(SPC)
            for r in res.results]
    return np.concatenate(outs).reshape(B, 1).astype(np.float32)



# revision 5
# speedup vs baseline: 2.5861x; 2.5861x over previous
"""DeepCrossing (embedding bag lookup + residual MLP) Trainium2 kernel.

Strategy (8 NeuronCores, data-parallel over batch):
  - Batch 4096 split 512 samples/core; the 1M x 64 embedding table is
    replicated per core as bf16 padded to 256B rows ([1M, 128] bf16).
  - Gather is 3-hop to dodge two hardware limits (indirect DMA = 1 offset
    per partition at ~1us/op; dma_gather is fast per-index but int16):
      hop1: per (vocab bank of 32768 rows, quarter of 128 samples), one
            dma_gather (bank-local int16 idx, slot-sorted, fixed 1024-slot
            capacity, trailing -1 padding which the SWDGE ucode strips)
      hop2: HWDGE strided write to DRAM scratch [31744, 128] bf16 per
            quarter (window < 32768 so hop3 idx fit int16)
      hop3: per (quarter, feature-pair) dma_gather from scratch in SLOT
            order -> [128 slots, 64] bf16 tiles
    Gathers use a raw InstDMAGatherAnt constructor: 128B payload with 256B
    row stride (elem_size=64 bf16, elem_step=128) — HW-validated.
  - dma_gather descriptor generation runs on one Q7 core-pair selected by
    queue_num (ucode: cpu_id/2 == queue_num). Gathers round-robin over 4
    SWDGE queues so 4 core pairs generate descriptors concurrently.
  - Pooling: PE matmuls out[64,16] = G.T @ S (S = 0/1 bag matrix) write the
    TRANSPOSED feature matrix featT [1664, 128] per quarter directly.
  - Residual MLP pipelined per quarter for blocks 0/1 (hides under gather);
    block 2 + final linear run 512-wide at the end with late-loaded weights
    (their SBUF slots rotate out of block 0's weight tiles).

Self-contained: hardcodes problem shapes from the task spec.
"""

import numpy as np
import ml_dtypes

# ---- problem constants ----
V = 1_000_000
D = 64
F = 26
L = 8
B = 4096
NCORES = 8
IN_DIM = F * D  # 1664
HIDDENS = (1024, 1024, 512)
P = 128
QS = 128                       # samples per quarter
BANK = 32768
QCAP = 1024                    # idx capacity per (bank, quarter)
BF16 = ml_dtypes.bfloat16


def _ceil(a, b):
    return -(-a // b)


# --------------------------------------------------------------------
# Host-side gather planning (per core); fixed capacities so one bass
# program serves every core.
# --------------------------------------------------------------------
def plan_gather(x_core, V_=V, F_=F, L_=L, SPC=None, bank=BANK, qcap=QCAP):
    SPC = SPC or x_core.shape[0]
    nbank = _ceil(V_, bank)
    nq = SPC // QS
    spq = F_ * QS * L_                        # slots per quarter
    xq = x_core.reshape(nq, QS, F_, L_)       # [q, s, f, l]
    ids = np.ascontiguousarray(xq.transpose(0, 2, 1, 3)).reshape(nq, spq)
    bk = ids // bank
    local = (ids % bank).astype(np.int16)

    # Pad each (quarter, bank) segment by repeating the last real index.
    # (Trailing -1 padding is NOT safe here: the ucode strips trailing
    # negatives and doorbells only the stripped descriptor count, while the
    # NX sequencer reserves ring space from num_idxs_reg — the mismatch
    # desyncs the SDMA tail pointer and executes stale descriptors.)
    flat1 = np.zeros(nq * nbank * qcap, dtype=np.int16)
    idx3 = np.zeros((nq, spq), dtype=np.int16)
    for q in range(nq):
        for b in range(nbank):
            sel = np.where(bk[q] == b)[0]
            n = len(sel)
            assert n <= qcap, f"bank overflow: {n} > {qcap}"
            o = (q * nbank + b) * qcap
            flat1[o:o + n] = local[q, sel]
            if n:
                flat1[o + n:o + qcap] = local[q, sel[-1]]
            idx3[q, sel] = (b * qcap + np.arange(n)).astype(np.int16)

    def wrap(flat):
        n = len(flat)
        w = np.zeros((16, n // 16), dtype=np.int16)
        w[np.arange(n) % 16, np.arange(n) // 16] = flat
        return np.ascontiguousarray(np.tile(w, (8, 1)))

    return {"idx1": wrap(flat1), "idx3": wrap(idx3.reshape(-1)),
            "nq": nq, "spq": spq, "nbank": nbank, "bank": bank, "qcap": qcap}


# --------------------------------------------------------------------
# Raw dma_gather: 128B payload + 256B stride (skips the %256 payload
# assert; stride encoding still requires multiples of 256B).
# --------------------------------------------------------------------
def _dma_gather_raw(gp, mybir, out_ap, in_ap, idxs_ap, num_idxs,
                    elem_size, elem_step, queue_num=0):
    assert idxs_ap.dtype == mybir.dt.int16
    assert in_ap.dtype == out_ap.dtype
    stride_bytes = elem_step * mybir.dt.size(in_ap.dtype)
    stride_bytes_256 = stride_bytes // 256
    assert stride_bytes_256 * 256 == stride_bytes and stride_bytes_256 < 256
    assert in_ap.ap[-1][1] == out_ap.ap[-1][1] == elem_size
    assert in_ap.ap[0][0] == elem_step
    _in_ap = gp.lower_ap_dma(in_ap, for_custom_bir_dma=True)
    _idxs_ap = gp.lower_ap(idxs_ap)
    _out_ap = gp.lower_ap(out_ap)
    return gp.add_instruction(
        mybir.InstDMAGatherAnt(
            name=gp.bass.get_next_instruction_name(),
            ins=[*_in_ap, _idxs_ap, gp.lower_val_access(gp.to_reg(num_idxs))],
            outs=[_out_ap],
            transpose=False,
            num_idxs=num_idxs,
            elem_size=elem_size,
            stride_bytes_256=stride_bytes_256,
            gen_mode=0,
            single_packet=False,
            queue_num=queue_num,
            sbuf_tokens_per_rank=0,
            sbuf_free_dim_per_rank=0,
            sbuf_free_dim_pad_per_rank=0,
            sbuf_byte_offset=0,
        ))


# --------------------------------------------------------------------
# Bass kernel builder
# --------------------------------------------------------------------
def build_nc(plan, V_=V, F_=F, SPC=B // NCORES, hiddens=HIDDENS):
    from contextlib import ExitStack

    from concourse import bacc, mybir, tile

    IN = F_ * D
    KT = IN // P                          # 13
    nq, nbank = plan["nq"], plan["nbank"]
    bank, qcap = plan["bank"], plan["qcap"]
    spq = plan["spq"]
    n1cols = plan["idx1"].shape[1]        # nq*nbank*qcap/16
    n3cols = plan["idx3"].shape[1]        # nq*spq/16
    q1cols = nbank * qcap // 16           # idx1 cols per quarter
    q3cols = spq // 16                    # idx3 cols per quarter
    scrr = nbank * qcap                   # scratch rows per quarter
    MTs = [h // P for h in hiddens]
    bf16 = mybir.dt.bfloat16
    f32 = mybir.dt.float32
    i16 = mybir.dt.int16
    AF = mybir.ActivationFunctionType
    ALU = mybir.AluOpType
    NQUEUES = 4

    nc = bacc.Bacc("TRN2", target_bir_lowering=False, debug=False,
                   num_swdge_queues=NQUEUES)

    emb = nc.dram_tensor("emb", [V_, 2 * D], bf16, kind="ExternalInput")
    idx1 = nc.dram_tensor("idx1", [P, n1cols], i16, kind="ExternalInput")
    idx3 = nc.dram_tensor("idx3", [P, n3cols], i16, kind="ExternalInput")
    S_in = nc.dram_tensor("S", [P, 16], bf16, kind="ExternalInput")
    w1d = [nc.dram_tensor(f"w1_{i}", [P, KT * h], bf16, kind="ExternalInput")
           for i, h in enumerate(hiddens)]
    w2d = [nc.dram_tensor(f"w2_{i}", [P, (h // P) * IN], bf16, kind="ExternalInput")
           for i, h in enumerate(hiddens)]
    b1d = nc.dram_tensor("b1", [P, sum(MTs)], f32, kind="ExternalInput")
    b2d = nc.dram_tensor("b2", [P, len(hiddens) * KT], f32, kind="ExternalInput")
    lwd = nc.dram_tensor("lin_w", [P, KT], bf16, kind="ExternalInput")
    lbd = nc.dram_tensor("lin_b", [1, 1], f32, kind="ExternalInput")
    out_d = nc.dram_tensor("out", [1, SPC], f32, kind="ExternalOutput")

    with tile.TileContext(nc) as tc, ExitStack() as ctx:
        const = ctx.enter_context(tc.tile_pool(name="const", bufs=1))
        i1p = ctx.enter_context(tc.tile_pool(name="i1", bufs=2))
        i3p = ctx.enter_context(tc.tile_pool(name="i3", bufs=2))
        g1pool = ctx.enter_context(tc.tile_pool(name="g1", bufs=8))
        g3pool = ctx.enter_context(tc.tile_pool(name="g3", bufs=4))
        wpool = ctx.enter_context(tc.tile_pool(name="w", bufs=2))
        apool = ctx.enter_context(tc.tile_pool(name="acts", bufs=1))
        hpool = ctx.enter_context(tc.tile_pool(name="hp", bufs=2))
        tpool = ctx.enter_context(tc.tile_pool(name="tmp", bufs=2))
        scrp = ctx.enter_context(tc.tile_pool(name="scr", bufs=1, space="DRAM"))
        fps_p = ctx.enter_context(tc.tile_pool(name="fps", bufs=2, space="PSUM"))
        mm_p = ctx.enter_context(tc.tile_pool(name="mm", bufs=2, space="PSUM"))
        mm2_p = ctx.enter_context(tc.tile_pool(name="mm2", bufs=2, space="PSUM"))
        op_p = ctx.enter_context(tc.tile_pool(name="op", bufs=1, space="PSUM"))

        S_sb = const.tile([P, 16], bf16)
        b1_sb = const.tile([P, sum(MTs)], f32)
        b2_sb = const.tile([P, len(hiddens) * KT], f32)
        lw_sb = const.tile([P, KT], bf16)
        lb_sb = const.tile([1, 1], f32)
        nc.sync.dma_start(out=S_sb[:], in_=S_in[:])
        nc.sync.dma_start(out=b1_sb[:], in_=b1d[:])
        nc.sync.dma_start(out=b2_sb[:], in_=b2d[:])
        nc.sync.dma_start(out=lw_sb[:], in_=lwd[:])
        nc.sync.dma_start(out=lb_sb[:], in_=lbd[:])

        featT = apool.tile([P, KT * SPC], bf16)
        out_sb = apool.tile([1, SPC], f32)

        # Blocks 0/1 weights resident up-front; block 2 tiles reuse the
        # rotated slots of block 0's and are loaded late (issued after the
        # last hop2 so their slot-free waits never block hop2 DMAs).
        w1_sb = []
        w2_sb = []
        for i in range(2):
            t1 = wpool.tile([P, KT * hiddens[i]], bf16, tag="w1",
                            padded_shape=[P, KT * hiddens[0]], name=f"w1s_{i}")
            nc.sync.dma_start(out=t1[:], in_=w1d[i][:])
            w1_sb.append(t1)
            t2 = wpool.tile([P, MTs[i] * IN], bf16, tag="w2",
                            padded_shape=[P, MTs[0] * IN], name=f"w2s_{i}")
            nc.scalar.dma_start(out=t2[:], in_=w2d[i][:])
            w2_sb.append(t2)

        scr = [scrp.tile([scrr, 2 * D], bf16, tag=f"scr{q}", name=f"scr{q}")
               for q in range(nq)]

        gq = [0]                            # SWDGE queue round-robin counter

        def next_q():
            q = gq[0] % NQUEUES
            gq[0] += 1
            return q

        hT = None
        for q in range(nq):
            # ---- stream this quarter's indices ----
            i1_sb = i1p.tile([P, q1cols], i16, tag="i1", name=f"i1_{q}")
            nc.sync.dma_start(out=i1_sb[:], in_=idx1[:, q * q1cols:(q + 1) * q1cols])
            i3_sb = i3p.tile([P, q3cols], i16, tag="i3", name=f"i3_{q}")
            nc.sync.dma_start(out=i3_sb[:], in_=idx3[:, q * q3cols:(q + 1) * q3cols])

            # ---- hop1 + hop2, per bank ----
            for b in range(nbank):
                bank_rows = min(bank, V_ - b * bank)
                g1 = g1pool.tile([P, qcap // P, D], bf16, tag="g1", name=f"g1_{q}_{b}")
                _dma_gather_raw(
                    nc.gpsimd, mybir, g1[:, :, :],
                    emb[b * bank: b * bank + bank_rows, 0:D],
                    i1_sb[:, b * (qcap // 16):(b + 1) * (qcap // 16)],
                    num_idxs=qcap, elem_size=D, elem_step=2 * D,
                    queue_num=next_q())
                # hop2 stays on sync: the scalar (Activation) queue carries
                # the MLP activations, which would head-of-line block these
                # writes (and thus g1 slot recycling) at quarter boundaries.
                dst = scr[q][b * qcap:(b + 1) * qcap, 0:D].rearrange(
                    "(blk p) d -> p blk d", p=P)
                nc.sync.dma_start(out=dst, in_=g1[:, :, :])

            # ---- hop3 + pooling ----
            CHUNK = 2 * QS * L              # 2048 slots = 2 features
            for kt in range(KT):
                g3 = g3pool.tile([P, CHUNK // P, D], bf16, tag="g3",
                                 name=f"g3_{q}_{kt}")
                _dma_gather_raw(
                    nc.gpsimd, mybir, g3[:, :, :], scr[q][:, 0:D],
                    i3_sb[:, kt * (CHUNK // 16):(kt + 1) * (CHUNK // 16)],
                    num_idxs=CHUNK, elem_size=D, elem_step=2 * D,
                    queue_num=next_q())
                fps = fps_p.tile([P, QS], f32, tag="fps", name=f"fps_{q}_{kt}")
                for blk in range(CHUNK // P):
                    f_loc = blk // 8
                    j = blk % 8
                    nc.tensor.matmul(
                        out=fps[f_loc * 64:(f_loc + 1) * 64,
                                j * 16:(j + 1) * 16],
                        lhsT=g3[:, blk, :],
                        rhs=S_sb[:],
                        start=True, stop=True)
                nc.vector.tensor_copy(
                    out=featT[:, kt * SPC + q * QS: kt * SPC + (q + 1) * QS],
                    in_=fps[:])

            # ---- residual blocks 0/1 on this quarter's 128 columns ----
            def fcol(k):
                return featT[:, k * SPC + q * QS: k * SPC + (q + 1) * QS]

            for i in range(2):
                h = hiddens[i]
                MT = MTs[i]
                hT = hpool.tile([P, MT * QS], bf16, tag="hT", name=f"hT_{q}_{i}")
                for m in range(MT):
                    ps = mm_p.tile([P, QS], f32, tag="mm", name=f"mmA_{q}_{i}_{m}")
                    for k in range(KT):
                        nc.tensor.matmul(
                            out=ps[:],
                            lhsT=w1_sb[i][:, k * h + m * P: k * h + (m + 1) * P],
                            rhs=fcol(k),
                            start=(k == 0), stop=(k == KT - 1))
                    nc.scalar.activation(
                        out=hT[:, m * QS:(m + 1) * QS], in_=ps[:], func=AF.Relu,
                        bias=b1_sb[:, i * MTs[0] + m: i * MTs[0] + m + 1])
                for k in range(KT):
                    ps = mm_p.tile([P, QS], f32, tag="mm", name=f"mmB_{q}_{i}_{k}")
                    for m in range(MT):
                        nc.tensor.matmul(
                            out=ps[:],
                            lhsT=w2_sb[i][:, m * IN + k * P: m * IN + (k + 1) * P],
                            rhs=hT[:, m * QS:(m + 1) * QS],
                            start=(m == 0), stop=(m == MT - 1))
                    tmp = tpool.tile([P, QS], f32, tag="tmp", name=f"tmp_{q}_{i}_{k}")
                    nc.vector.scalar_tensor_tensor(
                        out=tmp[:], in0=ps[:],
                        scalar=b2_sb[:, i * KT + k: i * KT + k + 1],
                        in1=fcol(k),
                        op0=ALU.add, op1=ALU.add)
                    nc.scalar.activation(out=fcol(k), in_=tmp[:], func=AF.Relu)

        # ---- block 2 weights (rotate into block 0's slots) ----
        t1 = wpool.tile([P, KT * hiddens[2]], bf16, tag="w1",
                        padded_shape=[P, KT * hiddens[0]], name="w1s_2")
        nc.sync.dma_start(out=t1[:], in_=w1d[2][:])
        w1_sb.append(t1)
        t2 = wpool.tile([P, MTs[2] * IN], bf16, tag="w2",
                        padded_shape=[P, MTs[0] * IN], name="w2s_2")
        nc.scalar.dma_start(out=t2[:], in_=w2d[2][:])
        w2_sb.append(t2)

        # ---- block 2 + final linear, full width ----
        h = hiddens[2]
        MT = MTs[2]
        hT2 = apool.tile([P, MT * SPC], bf16)
        for m in range(MT):
            ps = mm2_p.tile([P, SPC], f32, tag="mm2", name=f"mm2A_{m}")
            for k in range(KT):
                nc.tensor.matmul(
                    out=ps[:],
                    lhsT=w1_sb[2][:, k * h + m * P: k * h + (m + 1) * P],
                    rhs=featT[:, k * SPC:(k + 1) * SPC],
                    start=(k == 0), stop=(k == KT - 1))
            nc.scalar.activation(
                out=hT2[:, m * SPC:(m + 1) * SPC], in_=ps[:], func=AF.Relu,
                bias=b1_sb[:, 2 * MTs[0] + m: 2 * MTs[0] + m + 1])
        for k in range(KT):
            ps = mm2_p.tile([P, SPC], f32, tag="mm2", name=f"mm2B_{k}")
            for m in range(MT):
                nc.tensor.matmul(
                    out=ps[:],
                    lhsT=w2_sb[2][:, m * IN + k * P: m * IN + (k + 1) * P],
                    rhs=hT2[:, m * SPC:(m + 1) * SPC],
                    start=(m == 0), stop=(m == MT - 1))
            tmp = tpool.tile([P, SPC], f32, tag="tmp2", name=f"tmp2_{k}")
            nc.vector.scalar_tensor_tensor(
                out=tmp[:], in0=ps[:],
                scalar=b2_sb[:, 2 * KT + k: 2 * KT + k + 1],
                in1=featT[:, k * SPC:(k + 1) * SPC],
                op0=ALU.add, op1=ALU.add)
            nc.scalar.activation(
                out=featT[:, k * SPC:(k + 1) * SPC], in_=tmp[:], func=AF.Relu)

        ps = op_p.tile([1, SPC], f32, tag="op")
        for k in range(KT):
            nc.tensor.matmul(
                out=ps[:], lhsT=lw_sb[:, k:k + 1],
                rhs=featT[:, k * SPC:(k + 1) * SPC],
                start=(k == 0), stop=(k == KT - 1))
        nc.scalar.activation(out=out_sb[:], in_=ps[:], func=AF.Sigmoid,
                             bias=lb_sb[0:1, 0:1])
        nc.sync.dma_start(out=out_d[:], in_=out_sb[:])

    nc.compile()
    return nc


# --------------------------------------------------------------------
# Host-side input prep
# --------------------------------------------------------------------
def _prep_weights(inputs, hiddens=HIDDENS, F_=F):
    IN = F_ * D
    KT = IN // P
    MTs = [h // P for h in hiddens]
    embf = np.asarray(inputs["emb_table"], dtype=np.float32)
    embp = np.zeros((embf.shape[0], 2 * D), dtype=BF16)
    embp[:, :D] = embf.astype(BF16)
    shared = {
        "emb": embp,
        "S": (np.arange(P)[:, None] // L == np.arange(16)[None, :]).astype(BF16),
    }
    for i, h in enumerate(hiddens):
        w1 = np.asarray(inputs[f"w1_{i}"], dtype=np.float32)
        w2 = np.asarray(inputs[f"w2_{i}"], dtype=np.float32)
        shared[f"w1_{i}"] = np.ascontiguousarray(
            w1.reshape(KT, P, h).transpose(1, 0, 2).reshape(P, KT * h).astype(BF16))
        shared[f"w2_{i}"] = np.ascontiguousarray(
            w2.reshape(h // P, P, IN).transpose(1, 0, 2)
            .reshape(P, (h // P) * IN).astype(BF16))
    b1 = np.concatenate([np.asarray(inputs[f"b1_{i}"], dtype=np.float32)
                         .reshape(MTs[i], P).T for i in range(len(hiddens))],
                        axis=1)
    b2 = np.concatenate([np.asarray(inputs[f"b2_{i}"], dtype=np.float32)
                         .reshape(KT, P).T for i in range(len(hiddens))], axis=1)
    shared["b1"] = np.ascontiguousarray(b1)
    shared["b2"] = np.ascontiguousarray(b2)
    shared["lin_w"] = np.ascontiguousarray(
        np.asarray(inputs["lin_w"], dtype=np.float32).reshape(KT, P).T.astype(BF16))
    shared["lin_b"] = np.asarray(inputs["lin_b"], dtype=np.float32).reshape(1, 1)
    return shared


# --------------------------------------------------------------------
# Public entry point
# --------------------------------------------------------------------
_NC_CACHE = {}


def kernel(**inputs):
    from concourse.bass_utils import run_bass_kernel_spmd

    SPC = B // NCORES
    x = np.asarray(inputs["x"]).astype(np.int64)
    shared = _prep_weights(inputs)
    plans = [plan_gather(x[c * SPC:(c + 1) * SPC]) for c in range(NCORES)]
    if "nc" not in _NC_CACHE:
        _NC_CACHE["nc"] = build_nc(plans[0])
    nc = _NC_CACHE["nc"]
    in_maps = []
    for c in range(NCORES):
        m = dict(shared)
        m["idx1"] = plans[c]["idx1"]
        m["idx3"] = plans[c]["idx3"]
        in_maps.append(m)
    res = run_bass_kernel_spmd(nc, in_maps, core_ids=list(range(NCORES)))
    outs = [np.asarray(r["out"], dtype=np.float32).reshape(SPC)
            for r in res.results]
    return np.concatenate(outs).reshape(B, 1).astype(np.float32)
